# revision 45
# baseline (speedup 1.0000x reference)
"""Trainium2 Bass kernel for nn_LocalPointDecoder (sparse_attention).

Algorithm (per query point n):
  c[n]  = softmax_m(-|q_n - pp_m|^2 / VAR) @ fea          (Gaussian point attention)
  out[n] = MLP(c[n], q_n)                                  (5-block ResNet MLP, H=256)

Sharding: query points N=16384 split across 8 cores (2048 each); every core
holds the full pp/fea context and all MLP weights.

v2 design (from trace analysis of the v1 kernel):
  - d2^T via ONE fp16 matmul with K=15 hi/lo-split quadratic-form operands
    (replaces the fp32 LOW_HIGH matmul that ran at 1/3 rate and dominated the
    attention phase).  ppack/qpack split as p = p_hi + p_lo (fp16 each):
      d2 = ph.qh + ph.ql + pl.qh  (+ O(1e-6) dropped pl.ql term)
    3-way row-tiled (strips 0/32/64) matching 3-bank PSUM s-tiles.
  - exp on ACT in [128,1536] instructions (3 PSUM banks per ACTIVATE) to
    amortize the ~352-cycle ACT instruction overhead; fp16 weights out.
  - softmax denominator via col-tiled ones-matmuls (tile_position=(0,32g))
    accumulating [1,512] partials in PSUM strips 0/32/64 concurrently,
    replacing 87us of DVE tensor_adds.  The dn bank is DVE-memset to zero
    first and the strip matmuls use start=False (accumulate-or-overwrite on
    zero data is correct either way), so the three strips never clear each
    other's has_written bits.  Strip partials are combined by a K=65
    ones-matmul on the ACT-copied rows (cross-partition sum must be PE).
  - normalization: fast reciprocal approx on DVE, fp16 K=1 broadcast matmul,
    DVE multiply.
  - MLP: H on partitions, residual stream accumulates in PSUM; relu+bias on
    DVE via tensor_scalar(add, max) (PSUM-src), freeing ACT for the exps.
  - fp16 everywhere on-device (same PE rate as bf16, 8x better mantissa).
"""

import numpy as np

import concourse.bass as bass
import concourse.mybir as mybir
from concourse import bacc
import concourse.tile as tile
from concourse.bass_utils import run_bass_kernel_spmd

F32 = mybir.dt.float32
F16 = mybir.dt.float16
AF = mybir.ActivationFunctionType
ALU = mybir.AluOpType

N_CORES = 8
N, M, D, C, H = 16384, 4096, 3, 128, 256
NB = 5
NC_PER = N // N_CORES          # 2048 queries per core
CHUNK = 512                    # free-dim tile (one fp32 PSUM bank)
N_CHUNKS = NC_PER // CHUNK     # 4
MT = M // 128                  # 32 context tiles
NG = 11                        # m-tile groups per chunk: 10x3 + 1x2
INV_VAR = 100.0                # 1 / (0.1**2)
KQ = 15                        # hi/lo-split quadratic form contraction

# fp16 weight-table column offsets (one DMA for all matmul weights)
OFF_FCC = 0                      # [128, NB*H]   fc_c_W[i].T h-tiles
OFF_B0 = OFF_FCC + NB * H        # [128, NB*2*H] blk0_W[i].T (kt, ht)
OFF_B1 = OFF_B0 + NB * 2 * H
OFF_OW = OFF_B1 + NB * 2 * H     # [128, 2] out_W.T k-tiles
OFF_ONEC = OFF_OW + 2            # [128, 1] ones column (dn partition-sum lhsT)
OFF_ONER = OFF_ONEC + 1          # [128 cols] row 0 = ones (rb broadcast lhsT)
OFF_FCPW = OFF_ONER + 128        # rows 0-2: fc_p_W.T h-tiles
WCOLS = OFF_FCPW + H
# f32 table: relu bias vectors
OFF_BIAS = 0                     # [128, 22] bias vecs (ht*11 + v)
FCOLS = OFF_BIAS + 22

def build_bass(slot_sizes: tuple) -> bass.Bass:
    nc = bacc.Bacc()

    ngroups = [(sz + 2) // 3 for sz in slot_sizes]
    base_group = [sum(ngroups[:i]) for i in range(N_CHUNKS)]
    base_tile = [sum(slot_sizes[:i]) for i in range(N_CHUNKS)]
    total_groups = sum(ngroups)
    total_tiles = sum(slot_sizes)
    QW = NC_PER + total_groups * 128

    qpp_d = nc.declare_dram_parameter("qpp", [79, QW], F16, isOutput=False)
    fea_d = nc.declare_dram_parameter("fea", [total_tiles * 128, C], F16,
                                      isOutput=False)
    wtab_d = nc.declare_dram_parameter("wtab", [128, WCOLS], F16, isOutput=False)
    ftab_d = nc.declare_dram_parameter("ftab", [128, FCOLS], F32, isOutput=False)
    out_d = nc.declare_dram_parameter("out", [1, NC_PER], F32, isOutput=True)
    import os
    dbg_d = None
    if os.environ.get("K_DEBUG"):
        dbg_d = nc.declare_dram_parameter("dbg", [C, NC_PER], F16, isOutput=True)

    with tile.TileContext(nc) as tc:
        with tc.tile_pool(name="consts", bufs=1) as consts:
            qpp_sb = consts.tile([79, QW], F16, tag="qpp")
            nc.sync.dma_start(out=qpp_sb, in_=qpp_d[:, :])
            fea_sb = consts.tile([128, total_tiles, C], F16, tag="fea")
            fea_r = fea_d[:, :].rearrange("(t p) c -> p t c", p=128)
            fq_bounds = [round(fq * total_tiles / 4) for fq in range(5)]
            for fq in range(4):
                nc.scalar.dma_start(
                    out=fea_sb[:, fq_bounds[fq] : fq_bounds[fq + 1], :],
                    in_=fea_r[:, fq_bounds[fq] : fq_bounds[fq + 1], :],
                )
            wt_sb = consts.tile([128, WCOLS], F16, tag="wtab")
            nc.scalar.dma_start(out=wt_sb, in_=wtab_d[:, :])
            ft_sb = consts.tile([128, FCOLS], F32, tag="ftab")
            nc.scalar.dma_start(out=ft_sb, in_=ftab_d[:, :])

            fcc_lhsT = lambda i, ht: wt_sb[
                :, OFF_FCC + i * H + ht * 128 : OFF_FCC + i * H + ht * 128 + 128
            ]
            b0_lhsT = lambda i, kt, ht: wt_sb[
                :,
                OFF_B0 + i * 512 + kt * 256 + ht * 128 : OFF_B0
                + i * 512 + kt * 256 + ht * 128 + 128,
            ]
            b1_lhsT = lambda i, kt, ht: wt_sb[
                :,
                OFF_B1 + i * 512 + kt * 256 + ht * 128 : OFF_B1
                + i * 512 + kt * 256 + ht * 128 + 128,
            ]
            fcp_lhsT = lambda ht: wt_sb[0:D, OFF_FCPW + ht * 128 : OFF_FCPW + ht * 128 + 128]
            ow_lhsT = lambda kt: wt_sb[:, OFF_OW + kt : OFF_OW + kt + 1]
            bias_ap = lambda ht, v: ft_sb[:, OFF_BIAS + ht * 11 + v : OFF_BIAS + ht * 11 + v + 1]
            ones_col = wt_sb[:, OFF_ONEC : OFF_ONEC + 1]
            ones_row = wt_sb[0:1, OFF_ONER : OFF_ONER + 128]

            cn_sb = consts.tile([C, NC_PER], F16, tag="cn")  # normalized c^T

            # pp group-column base for (chunk, group) (stripe = 32*(idx%3))
            ppcol = lambda ch, j: NC_PER + (base_group[ch] + j) * 128

            # ---------------- attention phase ----------------
            with (
                tc.tile_pool(name="spsum", bufs=2, space="PSUM") as s_pool,
                tc.tile_pool(name="ctpsum", bufs=1, space="PSUM") as c_pool,
                tc.tile_pool(name="dnpsum", bufs=1, space="PSUM") as d_pool,
                tc.tile_pool(name="wsb", bufs=3) as w_pool,
                tc.tile_pool(name="nrm", bufs=2) as n_pool,
            ):
                for ch in range(N_CHUNKS):
                    nsl = slice(ch * CHUNK, (ch + 1) * CHUNK)
                    sz = slot_sizes[ch]
                    ngr = ngroups[ch]
                    obs_ps = None
                    if ch == 0:
                        obs_ps = c_pool.tile([1, 8], F32, tag="ct", name="obs")
                    ct_ps = c_pool.tile([C, CHUNK], F32, tag="ct")
                    acc_sb = n_pool.tile([128, CHUNK], F16, tag="acc")

                    def emit_d2(j, nsl=nsl, ch=ch, sz=sz):
                        s_ps = s_pool.tile([128, 3 * CHUNK], F32, tag="s")
                        pc = ppcol(ch, j)
                        for g in range(min(3, sz - 3 * j)):
                            nc.tensor.matmul(
                                s_ps[:, g * CHUNK : (g + 1) * CHUNK],
                                lhsT=qpp_sb[32 * g : 32 * g + KQ,
                                            pc : pc + 128],
                                rhs=qpp_sb[32 * g : 32 * g + KQ, nsl],
                                start=True,
                                stop=True,
                                tile_position=(32 * g, 0),
                            )
                        return s_ps

                    s_tiles = {0: emit_d2(0)}
                    if obs_ps is not None:
                        # absorb every fea-quarter DMA wait plus the wtab DMA
                        # wait into PE program order before the first c^T
                        # matmuls (subtile deps track each quarter
                        # separately).  Scratch target: the ct bank, fully
                        # overwritten by the idx=0 start=True matmul.
                        for fq in range(4):
                            nc.tensor.matmul(
                                obs_ps[0:1, fq : fq + 1],
                                lhsT=fea_sb[0:1, fq_bounds[fq], 0:1],
                                rhs=fea_sb[0:1, fq_bounds[fq], 0:1],
                                start=True, stop=True,
                            )
                        nc.tensor.matmul(
                            obs_ps[0:1, 4:5], lhsT=wt_sb[0:1, 0:1],
                            rhs=wt_sb[0:1, 0:1], start=True, stop=True,
                        )
                    for j in range(ngr):
                        s_ps = s_tiles.pop(j)
                        ng = min(3, sz - 3 * j)
                        w_sb = w_pool.tile([128, 3 * CHUNK], F16, tag="w")
                        nc.scalar.activation(
                            w_sb[:, : ng * CHUNK], s_ps[:, : ng * CHUNK],
                            AF.Exp, scale=-INV_VAR,
                        )
                        if j + 1 < ngr:
                            s_tiles[j + 1] = emit_d2(j + 1)
                        for g in range(ng):
                            idx = 3 * j + g
                            nc.tensor.matmul(
                                ct_ps,
                                lhsT=fea_sb[:, base_tile[ch] + idx, :],
                                rhs=w_sb[:, g * CHUNK : (g + 1) * CHUNK],
                                start=(idx == 0),
                                stop=(idx == sz - 1),
                                skip_group_check=True,
                            )
                        for g in range(ng):
                            # fp16 elementwise running sum of the w tiles on
                            # DVE (idle during attention); partition-reduced
                            # by one ones-matmul per chunk afterwards
                            if 3 * j + g == 0:
                                nc.vector.tensor_copy(
                                    acc_sb, w_sb[:, g * CHUNK : (g + 1) * CHUNK]
                                )
                            else:
                                nc.vector.tensor_add(
                                    acc_sb, acc_sb,
                                    w_sb[:, g * CHUNK : (g + 1) * CHUNK],
                                )

                    # free the ct bank early for the next chunk
                    ct_sb = n_pool.tile([C, CHUNK], F32, tag="ctc")
                    nc.scalar.activation(ct_sb, ct_ps, AF.Copy)
                    # partition-reduce the accumulated w sums, then the whole
                    # reciprocal/broadcast chain stays on DVE so both matmuls
                    # below carry a single (DVE) sem wait
                    ds_ps = d_pool.tile([1, CHUNK], F32, tag="dn")
                    nc.tensor.matmul(
                        ds_ps, lhsT=ones_col, rhs=acc_sb, start=True, stop=True,
                    )
                    ds_sb = n_pool.tile([1, CHUNK], F32, tag="ds")
                    nc.vector.tensor_copy(ds_sb, ds_ps)
                    r_sb = n_pool.tile([1, CHUNK], F32, tag="r")
                    nc.vector.reciprocal_approx_fast(out=r_sb, in_=ds_sb)
                    r16_sb = n_pool.tile([1, CHUNK], F16, tag="r16")
                    nc.vector.tensor_copy(r16_sb, r_sb)
                    # broadcast r to 128 partitions via K=1 ones-row matmul
                    rb_ps = d_pool.tile([128, CHUNK], F32, tag="dn")
                    nc.tensor.matmul(
                        rb_ps, lhsT=ones_row, rhs=r16_sb, start=True, stop=True
                    )
                    rb_sb = n_pool.tile([128, CHUNK], F16, tag="rb")
                    nc.vector.tensor_copy(rb_sb, rb_ps)
                    nc.vector.tensor_mul(cn_sb[:, nsl], ct_sb, rb_sb)

            if dbg_d is not None:
                nc.sync.dma_start(out=dbg_d[:, :], in_=cn_sb)

            # ---------------- MLP phase ----------------
            # net^T resident in PSUM per (ht, sub); blk1/fc_c matmuls
            # accumulate the residual stream in place.  Two sub-chunks in
            # flight: 2 ht x 2 sub net banks + 2 ht x 2 sub h banks = 8.
            # Relus alternate engines by sub parity (even subs DVE, odd subs
            # ACT, which is idle during the MLP) with parity-split PSUM tags
            # so every matmul's cross-engine deps stay on ONE engine counter.
            def relu_bias(parity, out, in_, bias):
                if parity == 0:
                    nc.vector.tensor_scalar(
                        out=out, in0=in_, scalar1=bias, scalar2=0.0,
                        op0=ALU.add, op1=ALU.max,
                    )
                else:
                    nc.scalar.activation(out, in_, AF.Relu, bias=bias)

            ry_keep = []
            with tc.tile_pool(name="rysb", bufs=8) as ry_pool:
                with (
                    tc.tile_pool(name="netpsum", bufs=2, space="PSUM") as np_pool,
                    tc.tile_pool(name="hpsum", bufs=2, space="PSUM") as h_pool,
                    tc.tile_pool(name="asb", bufs=4) as a_pool,
                    tc.tile_pool(name="bsb", bufs=4) as b_pool,
                ):
                    for g0 in range(0, N_CHUNKS, 2):
                        subs = (g0, g0 + 1)
                        net = {}
                        for sub in subs:
                            nsl = slice(sub * CHUNK, (sub + 1) * CHUNK)
                            for ht in range(2):
                                net_ps = np_pool.tile(
                                    [128, CHUNK], F32, tag=f"net{sub % 2}",
                                    name=f"net{sub % 2}_{ht}",
                                )
                                nc.tensor.matmul(
                                    net_ps, lhsT=fcp_lhsT(ht),
                                    rhs=qpp_sb[0:D, nsl],
                                    start=True, stop=False,
                                )
                                nc.tensor.matmul(
                                    net_ps, lhsT=fcc_lhsT(0, ht),
                                    rhs=cn_sb[:, nsl],
                                    start=False, stop=True,
                                )
                                net[(ht, sub)] = net_ps

                        for i in range(NB):
                            for sub in subs:
                                par = sub % 2
                                nsl = slice(sub * CHUNK, (sub + 1) * CHUNK)
                                rx = []
                                for ht in range(2):
                                    rx_sb = a_pool.tile(
                                        [128, CHUNK], F16, tag=f"rx{par}",
                                        name=f"rx{par}_{ht}",
                                    )
                                    relu_bias(par, rx_sb, net[(ht, sub)],
                                              bias_ap(ht, i))
                                    rx.append(rx_sb)
                                h_tiles = []
                                for ht in range(2):
                                    h_ps = h_pool.tile(
                                        [128, CHUNK], F32, tag=f"h{par}",
                                        name=f"h{par}_{ht}",
                                    )
                                    nc.tensor.matmul(
                                        h_ps, lhsT=b0_lhsT(i, 0, ht), rhs=rx[0],
                                        start=True, stop=False,
                                    )
                                    nc.tensor.matmul(
                                        h_ps, lhsT=b0_lhsT(i, 1, ht), rhs=rx[1],
                                        start=False, stop=True,
                                    )
                                    h_tiles.append(h_ps)
                                rh = []
                                for ht in range(2):
                                    rh_sb = b_pool.tile(
                                        [128, CHUNK], F16, tag=f"rh{par}",
                                        name=f"rh{par}_{ht}",
                                    )
                                    relu_bias(par, rh_sb, h_tiles[ht],
                                              bias_ap(ht, 6 + i))
                                    rh.append(rh_sb)
                                last = i == NB - 1
                                for ht in range(2):
                                    nc.tensor.matmul(
                                        net[(ht, sub)], lhsT=b1_lhsT(i, 0, ht),
                                        rhs=rh[0], start=False, stop=False,
                                        skip_group_check=True,
                                    )
                                    nc.tensor.matmul(
                                        net[(ht, sub)], lhsT=b1_lhsT(i, 1, ht),
                                        rhs=rh[1], start=False, stop=last,
                                        skip_group_check=True,
                                    )
                                    if not last:
                                        nc.tensor.matmul(
                                            net[(ht, sub)],
                                            lhsT=fcc_lhsT(i + 1, ht),
                                            rhs=cn_sb[:, nsl],
                                            start=False, stop=True,
                                            skip_group_check=True,
                                        )

                        # ry = relu(net + B_y), kept in SBUF for the out stage
                        for sub in subs:
                            rys = []
                            for ht in range(2):
                                ry_sb = ry_pool.tile(
                                    [128, CHUNK], F16, tag="ry",
                                    name=f"ry_{sub}_{ht}",
                                )
                                relu_bias(sub % 2, ry_sb, net[(ht, sub)],
                                          bias_ap(ht, 5))
                                rys.append(ry_sb)
                            ry_keep.append((sub, rys))

                # out = out_W @ ry   (+ out_b added on host)
                with (
                    tc.tile_pool(name="opsum", bufs=1, space="PSUM") as o_pool,
                    tc.tile_pool(name="osb", bufs=1) as os_pool,
                ):
                    for sub, rys in ry_keep:
                        par = sub % 2
                        nsl = slice(sub * CHUNK, (sub + 1) * CHUNK)
                        o_ps = o_pool.tile([1, CHUNK], F32, tag=f"o{par}",
                                           name=f"o{par}")
                        nc.tensor.matmul(
                            o_ps, lhsT=ow_lhsT(0), rhs=rys[0],
                            start=True, stop=False,
                        )
                        nc.tensor.matmul(
                            o_ps, lhsT=ow_lhsT(1), rhs=rys[1],
                            start=False, stop=True,
                        )
                        # copy engine matches the parity so the next out
                        # matmul in this bank waits on one engine only
                        out_sb = os_pool.tile([1, CHUNK], F32, tag=f"osb{par}",
                                              name=f"osb{par}")
                        if par == 0:
                            nc.vector.tensor_copy(out_sb, o_ps)
                        else:
                            nc.scalar.activation(out_sb, o_ps, AF.Copy)
                        nc.sync.dma_start(out=out_d[:, nsl], in_=out_sb)

    return nc


def _hilo(x):
    hi = x.astype(np.float16)
    lo = (x - hi.astype(np.float32)).astype(np.float16)
    return hi, lo


def _kd_sort(pts, splits):
    """Recursive exact-split spatial sort -> list of equal-size index boxes."""
    def rec(ids, depth):
        if depth == len(splits):
            return [ids]
        ax, k = splits[depth]
        order = ids[np.argsort(pts[ids, ax], kind="stable")]
        return [x for part in np.array_split(order, k)
                for x in rec(part, depth + 1)]
    return rec(np.arange(len(pts)), 0)


def _spatial_plan(p, pp):
    """Sort queries into 32 compact chunks of 512 and context into 32 tiles
    of 128; keep only (chunk, tile) pairs that can contribute; balance chunks
    across cores.  Returns (qperm, pperm, core_slots, slot_sizes) where
    core_slots[c][s] is the per-slot kept-tile list (tile ids into pperm
    tiles), padded later to slot_sizes[s]."""
    qboxes = _kd_sort(p, [(0, 4), (1, 4), (2, 2)])
    pboxes = _kd_sort(pp, [(0, 4), (1, 4), (2, 2)])
    qbb = [(p[b].min(0), p[b].max(0)) for b in qboxes]
    pbb = [(pp[b].min(0), pp[b].max(0)) for b in pboxes]
    gap2 = np.zeros((32, 32))
    for i, (qlo, qhi) in enumerate(qbb):
        for jj, (plo, phi) in enumerate(pbb):
            g = np.maximum(0.0, np.maximum(plo - qhi, qlo - phi))
            gap2[i, jj] = float((g * g).sum())
    # per-chunk denominator lower bound via nearest-neighbor distances
    try:
        from scipy.spatial import cKDTree
        nnd, _ = cKDTree(pp).query(p, k=1)
    except Exception:
        nnd = np.full(len(p), 0.15)  # conservative whp bound
    keeps = []
    for i in range(32):
        dlow = float(np.exp(-INV_VAR * (nnd[qboxes[i]] ** 2).max()))
        ub = 128.0 * np.exp(-INV_VAR * gap2[i])
        order = np.argsort(-gap2[i])
        dropped, kept = 0.0, set(range(32))
        for t in order:
            if gap2[i][t] == 0.0:
                break
            if dropped + ub[t] <= 1e-3 * dlow:
                dropped += ub[t]
                kept.discard(int(t))
            else:
                break
        keeps.append(sorted(kept))
    kc = np.array([len(k) for k in keeps])
    # LPT assignment of chunks to cores (4 each), then within-core
    # descending-size slot order; program slot size = max across cores
    order = np.argsort(-kc)
    cores, loads = [[] for _ in range(N_CORES)], [0] * N_CORES
    for ci in order:
        c = min(range(N_CORES),
                key=lambda c: (loads[c], len(cores[c]))
                if len(cores[c]) < N_CHUNKS else (1 << 30, 0))
        cores[c].append(int(ci))
        loads[c] += int(kc[ci])
    core_chunks = [sorted(cs, key=lambda x: -kc[x]) for cs in cores]
    slot_sizes = tuple(
        int(max(kc[core_chunks[c][s]] for c in range(N_CORES)))
        for s in range(N_CHUNKS)
    )
    qperm = np.concatenate(
        [qboxes[ci] for c in range(N_CORES) for ci in core_chunks[c]]
    )
    core_slots = [[keeps[ci] for ci in core_chunks[c]] for c in range(N_CORES)]
    return qperm, pboxes, core_slots, slot_sizes


def host_prep(inputs):
    p = np.asarray(inputs["p"], np.float32)[0]      # [N, 3]
    pp = np.asarray(inputs["pp"], np.float32)[0]    # [M, 3]
    fea = np.ascontiguousarray(np.asarray(inputs["fea"], np.float32)[0])  # [M, C]

    qperm, pboxes, core_slots, slot_sizes = _spatial_plan(p, pp)
    ngroups = [(sz + 2) // 3 for sz in slot_sizes]
    base_group = [sum(ngroups[:i]) for i in range(N_CHUNKS)]
    total_groups = sum(ngroups)
    total_tiles = sum(slot_sizes)
    QW = NC_PER + total_groups * 128

    ps = p[qperm]                                   # spatially sorted queries
    qpack = np.empty((5, N), np.float32)
    qpack[0:3] = ps.T
    qpack[3] = 1.0
    qpack[4] = (ps * ps).sum(1)
    ppack = np.empty((5, M), np.float32)
    ppack[0:3] = -2.0 * pp.T
    ppack[3] = (pp * pp).sum(1)
    ppack[4] = 1.0

    qh, ql = _hilo(qpack)
    ph, pl = _hilo(ppack)
    # K=15 contraction: lhsT rows [ph; ph; pl], rhs rows [qh; ql; qh]
    q15 = np.concatenate([qh, ql, qh], axis=0)      # [15, N]
    p15 = np.concatenate([ph, ph, pl], axis=0)      # [15, M]
    # padding pseudo-tile: d2 = |q|^2 + 8 -> exp underflows to exactly 0
    pad15 = np.zeros((KQ, 128), np.float16)
    pad15[3] = 8.0
    pad15[4] = 1.0
    pad15[8] = 8.0
    pad15[9] = 1.0

    fc_p_W = np.asarray(inputs["fc_p_W"], np.float32)    # [H, 3]
    fc_c_W = np.asarray(inputs["fc_c_W"], np.float32)    # [NB, H, C]
    blk0_W = np.asarray(inputs["blk0_W"], np.float32)    # [NB, H, H]
    blk1_W = np.asarray(inputs["blk1_W"], np.float32)
    out_W = np.asarray(inputs["out_W"], np.float32)      # [1, H]
    fc_p_b = np.asarray(inputs["fc_p_b"], np.float32)
    fc_c_b = np.asarray(inputs["fc_c_b"], np.float32)    # [NB, H]
    blk0_b = np.asarray(inputs["blk0_b"], np.float32)
    blk1_b = np.asarray(inputs["blk1_b"], np.float32)
    out_b = float(np.asarray(inputs["out_b"], np.float32)[0])

    wtab = np.zeros((128, WCOLS), np.float16)
    wtab[:, OFF_FCC : OFF_FCC + NB * H] = fc_c_W.transpose(2, 0, 1).reshape(C, NB * H)
    wtab[:, OFF_B0 : OFF_B0 + NB * 2 * H] = (
        blk0_W.reshape(NB, H, 2, 128).transpose(3, 0, 2, 1).reshape(128, NB * 2 * H)
    )
    wtab[:, OFF_B1 : OFF_B1 + NB * 2 * H] = (
        blk1_W.reshape(NB, H, 2, 128).transpose(3, 0, 2, 1).reshape(128, NB * 2 * H)
    )
    wtab[:, OFF_OW : OFF_OW + 2] = out_W.reshape(2, 128).T
    wtab[:, OFF_ONEC] = 1.0
    wtab[0, OFF_ONER : OFF_ONER + 128] = 1.0
    wtab[0:D, OFF_FCPW : OFF_FCPW + H] = fc_p_W.T

    ftab = np.zeros((128, FCOLS), np.float32)
    # cumulative bias vectors folded into the relus:
    #   vec 0..4  = B_i  (bias of net before block i's first relu)
    #   vec 5     = B_y  (bias of net before the final relu)
    #   vec 6..10 = blk0_b[i]  (bias of h before block i's second relu)
    vecs = np.zeros((11, H), np.float32)
    B = fc_p_b + fc_c_b[0]
    for i in range(NB):
        vecs[i] = B
        vecs[6 + i] = blk0_b[i]
        B = B + blk1_b[i] + (fc_c_b[i + 1] if i + 1 < NB else 0.0)
    vecs[5] = B
    ftab[:, OFF_BIAS : OFF_BIAS + 22] = (
        vecs.reshape(11, 2, 128).transpose(2, 1, 0).reshape(128, 22)
    )

    # per-core packing: kept context tiles gathered per (chunk slot, group);
    # pp stripes at partitions 32*(idx%3)..+15, fea tiles in slot order
    fea16 = fea.astype(np.float16)
    shared = {"wtab": wtab, "ftab": ftab}
    in_maps = []
    for c in range(N_CORES):
        m = dict(shared)
        qc = q15[:, c * NC_PER : (c + 1) * NC_PER]   # [15, NC_PER]
        qg = np.zeros((79, QW), np.float16)
        for g in range(3):
            qg[32 * g : 32 * g + KQ, 0:NC_PER] = qc
        feap = np.zeros((total_tiles, 128, C), np.float16)
        toff = 0
        for s in range(N_CHUNKS):
            tiles = list(core_slots[c][s]) + [-1] * (slot_sizes[s] - len(core_slots[c][s]))
            for idx, t in enumerate(tiles):
                g, j = idx % 3, idx // 3
                col = NC_PER + (base_group[s] + j) * 128
                if t < 0:
                    qg[32 * g : 32 * g + KQ, col : col + 128] = pad15
                else:
                    ids = pboxes[t]
                    qg[32 * g : 32 * g + KQ, col : col + 128] = p15[:, ids]
                    feap[toff + idx] = fea16[ids]
            toff += slot_sizes[s]
        m["qpp"] = qg
        m["fea"] = feap.reshape(total_tiles * 128, C)
        in_maps.append(m)
    return in_maps, out_b, qperm, slot_sizes


_NC_CACHE = {}


def kernel(**inputs) -> np.ndarray:
    in_maps, out_b, qperm, slot_sizes = host_prep(inputs)
    if slot_sizes not in _NC_CACHE:
        nc = build_bass(slot_sizes)
        nc.finalize()
        _NC_CACHE[slot_sizes] = nc
    nc = _NC_CACHE[slot_sizes]
    res = run_bass_kernel_spmd(nc, in_maps, list(range(N_CORES)))
    parts = [res.results[c]["out"] for c in range(N_CORES)]
    out_sorted = np.concatenate(parts, axis=1)[0].astype(np.float32)
    out = np.empty((1, N), np.float32)
    out[0, qperm] = out_sorted + np.float32(out_b)
    return out
